# revision 15
# baseline (speedup 1.0000x reference)
"""Trainium2 Bass kernel for nn_Cross_Attention (B=16, C=256, H=W=96).

reference:
    q = Z1.reshape(B, C, N); k = Zr.reshape(B, C, N)         # N = H*W
    energy    = q @ k^T                                       # [B, C, C]
    attention = softmax(rowmax(energy) - energy, axis=-1)
    out       = attention @ k                                 # [B, C, N]
    return beta * out + Zr

Strategy: data-parallel over batch, 2 batches per NeuronCore on 8 cores.
Uploads per core: q^T in bf16 (host pre-packed [P, NT, C] partition-major so
the N-contraction matmul streams straight from DRAM) and Zr in f32.  k is
derived on-chip: kb = bf16(Zr) (ScalarE downcast) feeds the second matmul
directly and is transposed tile-by-tile on the TensorE (transpose-mode
matmul) into k^T tiles for the energy matmul — so k crosses HBM once.
softmax(max - e) == exp(min - e) / sum(exp(min - e)) row-wise: only a
row-min is needed, exp args are always <= 0 (no overflow), the sum is >= 1
(no div-by-0).  beta and 1/sum are folded into the attention weights before
the second matmul, so the final blend is a single add with the f32-resident
Zr (bitwise-exact output when beta == 0).
"""

from contextlib import ExitStack

import ml_dtypes
import numpy as np

import concourse.bass as bass
import concourse.tile as tile
from concourse import bacc, mybir
from concourse.bass_utils import run_bass_kernel_spmd
from concourse.masks import make_identity

B, C, H, W = 16, 256, 96, 96
N = H * W                    # 9216
P = 128
NCORES = 8
BL = B // NCORES             # 2 batches per core
CT = C // P                  # 2 c-tiles of 128
NT = N // P                  # 72 contraction tiles for energy
TCH = 18                     # qt tiles per DMA chunk
NCH = NT // TCH              # 4 chunks
NH = N // 2                  # 4608: kb slice width (half a c-tile row)
NQ = N // 4                  # 2304: zr tile width (quarter c-tile row)
TPH = NH // P                # 36 n-tiles per h-half
OW = 384                     # mm2 psum chunk width (6 per zr quarter)
WPH = NH // OW               # 12 psum chunks per h-half

F32 = mybir.dt.float32
BF16 = mybir.dt.bfloat16


def _build_program():
    nc = bacc.Bacc("TRN2", target_bir_lowering=False, debug=False,
                   num_devices=NCORES)

    qt_ext = nc.dram_tensor("qt", [BL, P, NT, C], BF16, kind="ExternalInput")
    zr_ext = nc.dram_tensor("zr", [BL, C, N], F32, kind="ExternalInput")
    beta_ext = nc.dram_tensor("beta", [1], F32, kind="ExternalInput")
    out_ext = nc.dram_tensor("out", [BL, C, N], F32, kind="ExternalOutput")

    with tile.TileContext(nc) as tc, ExitStack() as ctx:
        qtp = ctx.enter_context(tc.tile_pool(name="qtp", bufs=2))
        zrp = ctx.enter_context(tc.tile_pool(name="zrp", bufs=12))
        kbp = ctx.enter_context(tc.tile_pool(name="kbp", bufs=6))
        kttp = ctx.enter_context(tc.tile_pool(name="kttp", bufs=4))
        expp = ctx.enter_context(tc.tile_pool(name="expp", bufs=2))
        attp = ctx.enter_context(tc.tile_pool(name="attp", bufs=2))
        atTp = ctx.enter_context(tc.tile_pool(name="atTp", bufs=2))
        statp = ctx.enter_context(tc.tile_pool(name="statp", bufs=8))
        singles = ctx.enter_context(tc.tile_pool(name="singles", bufs=1))
        engp = ctx.enter_context(tc.tile_pool(name="engp", bufs=2, space="PSUM"))
        trp = ctx.enter_context(tc.tile_pool(name="trp", bufs=4, space="PSUM"))
        outp = ctx.enter_context(tc.tile_pool(name="outp", bufs=2, space="PSUM"))

        ident = singles.tile([P, P], BF16)
        make_identity(nc, ident)
        beta_sb = singles.tile([P, 1], F32)
        nc.gpsimd.dma_start(out=beta_sb, in_=beta_ext.ap().to_broadcast((P, 1)))

        for b in range(BL):
            # ---- interleaved load/compute pipeline: chunk i of the
            # energy matmul consumes zr quarter i (via the kb downcast and
            # PE transposes) and qt chunk i, so the sync-ring order
            # [zr(.,qi), qt_i] feeds compute just-in-time ----
            zr_tiles = {}
            kb = {}
            eng = [engp.tile([P, C], F32, name="eng") for _ in range(CT)]
            for i in range(NCH):
                h, qq = divmod(i, 2)
                for cj in range(CT):
                    zt = zrp.tile([P, NQ], F32)
                    nc.sync.dma_start(
                        out=zt,
                        in_=zr_ext[b, cj * P:(cj + 1) * P, i * NQ:(i + 1) * NQ],
                    )
                    zr_tiles[cj, i] = zt
                for cj in range(CT):
                    if qq == 0:
                        kb[cj, h] = kbp.tile([P, NH], BF16, name="kb_t")
                    nc.scalar.copy(out=kb[cj, h][:, qq * NQ:(qq + 1) * NQ],
                                   in_=zr_tiles[cj, i])
                qt_t = qtp.tile([P, TCH, C], BF16)
                nc.sync.dma_start(out=qt_t, in_=qt_ext[b, :, i * TCH:(i + 1) * TCH, :])
                # transpose+copy producers, then this chunk's matmuls
                ktts = []
                for tg in range(TCH // 4):
                    tr4 = trp.tile([P, 4, CT, P], BF16, name="tr4")
                    for tq in range(4):
                        t = i * TCH + tg * 4 + tq
                        th = t - h * TPH
                        for dj in range(CT):
                            nc.tensor.transpose(tr4[:, tq, dj, :],
                                                kb[dj, h][:, th * P:(th + 1) * P],
                                                ident)
                    ktt4 = kttp.tile([P, 4, CT * P], BF16, name="ktt4")
                    nc.scalar.copy(out=ktt4, in_=tr4)
                    ktts.extend(ktt4[:, tq, :] for tq in range(4))
                for tl in range(TCH // 4 * 4, TCH):
                    t = i * TCH + tl
                    th = t - h * TPH
                    tr2 = trp.tile([P, 4, CT, P], BF16, name="tr2", tag="tr4")
                    for dj in range(CT):
                        nc.tensor.transpose(tr2[:, 0, dj, :],
                                            kb[dj, h][:, th * P:(th + 1) * P],
                                            ident)
                    ktt1 = kttp.tile([P, 4, CT * P], BF16, name="ktt1", tag="ktt4")
                    nc.scalar.copy(out=ktt1[:, 0, :], in_=tr2[:, 0, :, :])
                    ktts.append(ktt1[:, 0, :])
                for tl in range(TCH):
                    t = i * TCH + tl
                    for ci in range(CT):
                        nc.tensor.matmul(
                            eng[ci],
                            lhsT=qt_t[:, tl, ci * P:(ci + 1) * P],
                            rhs=ktts[tl],
                            start=(t == 0),
                            stop=(t == NT - 1),
                        )

            # ---- softmax(max-e) = exp(min-e)/sum; fold beta/sum in ----
            attnT = atTp.tile([P, CT, C], BF16)
            for ci in range(CT):
                mn = statp.tile([P, 1], F32)
                nc.vector.tensor_reduce(out=mn, in_=eng[ci],
                                        axis=mybir.AxisListType.X,
                                        op=mybir.AluOpType.min)
                ex = expp.tile([P, C], F32)
                sm = statp.tile([P, 1], F32)
                nc.scalar.activation(out=ex, in_=eng[ci],
                                     func=mybir.ActivationFunctionType.Exp,
                                     bias=mn, scale=-1.0, accum_out=sm)
                rc = statp.tile([P, 1], F32)
                nc.vector.reciprocal(out=rc, in_=sm)
                rb = statp.tile([P, 1], F32)
                nc.vector.tensor_mul(out=rb, in0=rc, in1=beta_sb)
                at = attp.tile([P, C], BF16)
                nc.vector.tensor_scalar_mul(out=at, in0=ex, scalar1=rb)
                trA = trp.tile([P, CT, P], BF16, name="trA", tag="tr4")
                for dj in range(CT):
                    nc.tensor.transpose(trA[:, dj, :], at[:, dj * P:(dj + 1) * P],
                                        ident)
                nc.vector.tensor_copy(out=attnT[:, :, ci * P:(ci + 1) * P],
                                      in_=trA)

            # ---- out = attn @ k, blended in place into zr, streamed out ----
            # h-outer so the n-low half's stores launch while later work
            # streams; each 4608-wide slice is stored in two 2304-wide pieces
            for h in range(2):
                for ci in range(CT):
                    for qq in range(2):
                        q = h * 2 + qq
                        zt = zr_tiles[ci, q]
                        for wq in range(WPH // 2):
                            w = qq * (WPH // 2) + wq
                            ps = outp.tile([P, OW], F32)
                            for dj in range(CT):
                                nc.tensor.matmul(
                                    ps,
                                    lhsT=attnT[:, dj, ci * P:(ci + 1) * P],
                                    rhs=kb[dj, h][:, w * OW:(w + 1) * OW],
                                    start=(dj == 0),
                                    stop=(dj == CT - 1),
                                )
                            nc.vector.tensor_add(
                                out=zt[:, wq * OW:(wq + 1) * OW],
                                in0=ps,
                                in1=zt[:, wq * OW:(wq + 1) * OW])
                        nc.gpsimd.dma_start(
                            out=out_ext[b, ci * P:(ci + 1) * P,
                                        q * NQ:(q + 1) * NQ],
                            in_=zt,
                        )

    nc.compile()
    return nc


_NC_CACHE = None


def _get_program():
    global _NC_CACHE
    if _NC_CACHE is None:
        _NC_CACHE = _build_program()
    return _NC_CACHE


def pack_qt(Z1):
    # [B, C, N] -> bf16 [B, P, NT, C] with qt[b, p, t, c] = q[b, c, t*128+p]
    x = Z1.reshape(B, C, NT, P).astype(ml_dtypes.bfloat16)
    return np.ascontiguousarray(x.transpose(0, 3, 2, 1))


def kernel(Z1, Zr, beta):
    Z1 = np.asarray(Z1, dtype=np.float32)
    Zr = np.asarray(Zr, dtype=np.float32)
    beta = np.asarray(beta, dtype=np.float32).reshape(1)

    qt = pack_qt(Z1)
    zr = np.ascontiguousarray(Zr.reshape(B, C, N))

    in_maps = []
    for i in range(NCORES):
        s = slice(i * BL, (i + 1) * BL)
        in_maps.append({"qt": qt[s], "zr": zr[s], "beta": beta})

    nc = _get_program()
    res = run_bass_kernel_spmd(nc, in_maps, list(range(NCORES)))
    out = np.concatenate([r["out"] for r in res.results], axis=0)
    return out.reshape(B, C, H, W)
